# revision 30
# baseline (speedup 1.0000x reference)
"""Grouped gated DeltaNet (KDA-style) on 8 TRN2 NeuronCores — v2.

Sharding: core c -> (batch b = c//4, head-group hg = c%4 of 4 heads).

v2 restructure vs baseline:
- no act-table thrash: recurrence loop uses only Exp/Copy/Square (one set);
  RMS-norm Ln/Exp deferred to a single post-loop pass
- group k/q correlations via partition-sliced matmuls (no masked-k copies)
- decay-difference exp: PE broadcast (fp32) + 8 bias-fused Act exps; clamp
  fused into the weighting multiply (min(1,e)*pall)
- group-weighted reduction on PE (one-hot accumulating matmuls)
- triangular solve: Neumann doubling (5 levels), closed PSUM groups
- small elementwise work offloaded to the GpSimd engine
- quad (4-head) tiles for wide low-overhead ops
- cross-chunk software pipelining: next chunk's state-independent work
  interleaved into the current chunk's solve

Self-contained: B=2, T=1024, D=2048, H=16, DK=DV=128 hardcoded.
"""
import sys
sys.path.insert(0, '/opt/trn_rl_repo')
import numpy as np
import ml_dtypes
from contextlib import ExitStack

B, T, D = 2, 1024, 2048
H, DK, DV, GG = 16, 128, 128, 16
NG = DK // GG          # 8 gate groups per head
NH = 4                 # heads per core
C = 128                # chunk length
NCH = T // C
NLEV = 4               # Neumann doubling levels (covers N^k, k < 2^NLEV)
SCALE = DK ** -0.5
EPS = 1e-5

BF = ml_dtypes.bfloat16
_CACHE = {}


def _build():
    import concourse.tile as tile
    from concourse import bacc, mybir

    fp32 = mybir.dt.float32
    bf16 = mybir.dt.bfloat16
    Alu = mybir.AluOpType
    Act = mybir.ActivationFunctionType

    nc = bacc.Bacc("TRN2", target_bir_lowering=False, debug=False, num_devices=8)
    dp = lambda n, sh, dt: nc.dram_tensor(n, sh, dt, kind="ExternalInput").ap()
    hT = dp("hT", [D, T], bf16)
    wqkvg = dp("wqkvg", [D, 4 * NH * DK], bf16)
    wo = dp("wo", [NH * DV, D], bf16)
    wf1 = dp("wf1", [D, DV], bf16)
    wf2 = dp("wf2", [DV, NH * NG], bf16)
    wb = dp("wb", [D, NH], bf16)
    cw = dp("cw", [NH * DK, 12], fp32)
    nega = dp("nega", [NG, NH], fp32)
    dtb = dp("dtb", [NG, NH], fp32)
    bgc = dp("bgc", [DV, NH], fp32)
    normw = dp("normw", [DV, 1], fp32)
    repl = dp("repl", [NG, DK], bf16)
    self8f = dp("self8f", [NG, NG * C], bf16)
    onescol = dp("onescol", [DK, 1], bf16)
    oh4 = dp("oh4", [DK, 16], bf16)
    oh4b = dp("oh4b", [4, 4 * DK], bf16)
    evod = dp("evod", [DK, 4], fp32)
    oh8 = dp("oh8", [DK, 64], bf16)
    sel8b = dp("sel8b", [8, 8 * 128], bf16)
    sc8 = dp("sc8", [8, 1], fp32)
    eps8 = dp("eps8", [8, 1], fp32)
    nmaskM = dp("nmaskM", [C, C], bf16)   # -1 strictly upper (s<t)
    maskG = dp("maskG", [C, C], bf16)     # +1 upper incl diag (s<=t)
    idbf = dp("idbf", [128, 128], bf16)
    id4f = dp("id4f", [128, 4 * 128], bf16)
    idf32 = dp("idf32", [128, 128], fp32)
    outT = nc.dram_tensor("outT", [D, T], fp32, kind="ExternalOutput").ap()
    DBG = bool(__import__('os').environ.get('K2_DEBUG'))
    if DBG:
        dbg = {n: nc.dram_tensor(n, [128, T], fp32, kind="ExternalOutput").ap()
               for n in ['d_qb', 'd_kb', 'd_vb', 'd_gateb', 'd_yf']}
        dbg['d_gna'] = nc.dram_tensor('d_gna', [8, T], fp32,
                                      kind="ExternalOutput").ap()
        dbg['d_bsg'] = nc.dram_tensor('d_bsg', [4, T], fp32,
                                      kind="ExternalOutput").ap()
        dbg['d_sqs'] = nc.dram_tensor('d_sqs', [4, T], fp32,
                                      kind="ExternalOutput").ap()
        dbg['d_eall'] = nc.dram_tensor('d_eall', [128, NG * C], bf16,
                                       kind="ExternalOutput").ap()
        for n in ['d_H0', 'd_GtM', 'd_ub', 'd_cN']:
            dbg[n] = nc.dram_tensor(n, [128, C], fp32,
                                    kind="ExternalOutput").ap()

    with tile.TileContext(nc) as tc, ExitStack() as ctx:
        pool = lambda name, bufs, space="SBUF": ctx.enter_context(
            tc.tile_pool(name=name, bufs=bufs, space=space))

        cons = pool("cons", 1)
        htp = pool("htp", 1)
        wst = pool("wst", 1)
        wsm = pool("wsm", 1)
        pers = pool("pers", 1)
        convp = pool("convp", 1)
        chk = pool("chk", 2)
        st = pool("st", 1)

        dma = nc.sync.dma_start

        # ---- consts ----
        cwt = []
        for m in range(4):
            t = cons.tile([128, 12], fp32, tag=f"cw{m}", name=f"cw{m}")
            dma(t[:], cw[m * 128:(m + 1) * 128, :])
            cwt.append(t)

        def ctile(shape, dt, src, nm):
            t = cons.tile(shape, dt, tag=nm, name=nm)
            dma(t[:], src[:])
            return t
        negat = ctile([8, 4], fp32, nega, "negat")
        dtbt = ctile([8, 4], fp32, dtb, "dtbt")
        bgt = ctile([128, 4], fp32, bgc, "bgt")
        nwt = ctile([128, 1], fp32, normw, "nwt")
        replt = ctile([8, 128], bf16, repl, "replt")
        s8f = ctile([NG, NG * C], bf16, self8f, "s8f")
        oct_ = ctile([128, 1], bf16, onescol, "oct")
        oh4t = ctile([128, 16], bf16, oh4, "oh4t")
        oh4bt = ctile([4, 4 * 128], bf16, oh4b, "oh4bt")
        evodt = ctile([128, 4], fp32, evod, "evodt")
        oh8t = ctile([128, 64], bf16, oh8, "oh8t")
        s8b = ctile([8, 8 * 128], bf16, sel8b, "s8b")
        sc8t = ctile([8, 1], fp32, sc8, "sc8t")
        eps8t = ctile([8, 1], fp32, eps8, "eps8t")
        nmM = ctile([128, 128], bf16, nmaskM, "nmM")
        mGt = ctile([128, 128], bf16, maskG, "mGt")
        idb = ctile([128, 128], bf16, idbf, "idb")
        id4 = ctile([128, 4 * 128], bf16, id4f, "id4")
        idf = ctile([128, 128], fp32, idf32, "idf")
        ones8 = cons.tile([8, C], fp32, tag="ones8", name="ones8")
        nc.vector.memset(ones8[:], 1.0)
        eps4 = cons.tile([4, 1], fp32, tag="eps4", name="eps4")
        nc.vector.memset(eps4[:], EPS)
        neg4c = cons.tile([128, 4 * C], bf16, tag="neg4c", name="neg4c")
        nc.vector.memset(neg4c[:], -1.0)

        # ---- persistent activations ----
        mk = lambda p, nm, dt=bf16, sh=None: [
            p.tile(sh or [128, T], dt, tag=f"{nm}{m}", name=f"{nm}{m}")
            for m in range(4)]
        qb, kb, vb = mk(pers, "qb"), mk(pers, "kb"), mk(pers, "vb")
        gateb, yfall = mk(pers, "gateb"), mk(pers, "yfall")
        gnah = [cons.tile([8, T], fp32, tag=f"gna{h}", name=f"gna{h}")
                for h in range(4)]
        bsg = cons.tile([4, T], fp32, tag="bsg", name="bsg")
        sqs = cons.tile([4, T], fp32, tag="sqs", name="sqs")

        # =========== PHASE 1: projections ===========
        with tc.tile_pool(name="pj", bufs=1, space="PSUM") as pj, \
             tc.tile_pool(name="p1s", bufs=1) as p1s:
            ht = []
            for k in range(16):
                t = p1s.tile([128, T], bf16, tag=f"ht{k}", name=f"ht{k}")
                dma(t[:], hT[k * 128:(k + 1) * 128, :])
                ht.append(t)
            ssqp = [pj.tile([8, 512], fp32, tag=f"ssq{hf}", name=f"ssq{hf}")
                    for hf in range(2)]

            def load_w(proj):
                wt = [p1s.tile([128, 512], bf16, tag=f"w{k}", name=f"wt{k}")
                      for k in range(16)]
                for k in range(16):
                    dma(wt[k][:], wqkvg[k * 128:(k + 1) * 128,
                                        proj * 512:(proj + 1) * 512])
                return wt

            def project(wt, m, dst_bf16=None, conv_slot=None, pair=None,
                        gate_bias=None):
                xpad = None
                if conv_slot is not None:
                    xpad = p1s.tile([128, T + 3], fp32, tag="xpad",
                                      name="xpad", bufs=2)
                    nc.vector.memset(xpad[:, 0:3], 0.0)
                for half in range(2):
                    ps = pj.tile([128, 512], fp32, tag="ps512", name="projps",
                                 bufs=2)
                    for k in range(16):
                        nc.tensor.matmul(ps[:], wt[k][:, m * 128:(m + 1) * 128],
                                         ht[k][:, half * 512:(half + 1) * 512],
                                         start=(k == 0), stop=(k == 15))
                    if xpad is not None:
                        nc.scalar.copy(xpad[:, 3 + half * 512:
                                            3 + (half + 1) * 512], ps[:])
                    elif gate_bias is not None:
                        nc.scalar.activation(
                            dst_bf16[:, half * 512:(half + 1) * 512],
                            ps[:], Act.Silu, bias=gate_bias)
                    else:
                        nc.scalar.copy(dst_bf16[:, half * 512:(half + 1) * 512],
                                       ps[:])
                if xpad is None:
                    return
                # depthwise causal conv on GpSimd (Pool) engine
                cwm = cwt[m]
                s = conv_slot * 4
                a = p1s.tile([128, T], fp32, tag="acca", name="acca")
                bt = p1s.tile([128, T], fp32, tag="accb", name="accb")
                nc.vector.tensor_scalar(a[:], xpad[:, 3:3 + T],
                                        cwm[:, s + 3:s + 4], None, op0=Alu.mult)
                cur, nxt = a, bt
                for kk in (2, 1, 0):
                    nc.vector.scalar_tensor_tensor(
                        nxt[:], xpad[:, kk:kk + T], cwm[:, s + kk:s + kk + 1],
                        cur[:], op0=Alu.mult, op1=Alu.add)
                    cur, nxt = nxt, cur
                if pair is None:
                    nc.scalar.activation(dst_bf16[:], cur[:], Act.Silu)
                else:
                    qsil = qb[pair] if pair < 4 else kb[pair - 4]
                    nc.scalar.activation(qsil[:], cur[:], Act.Silu)
                    sq = p1s.tile([128, T], bf16, tag="sq", name="sq", bufs=2)
                    nc.vector.tensor_tensor(sq[:], qsil[:], qsil[:], op=Alu.mult)
                    for half in range(2):
                        nc.tensor.matmul(ssqp[half][:],
                                         oh8t[:, pair * 8:pair * 8 + 8],
                                         sq[:, half * 512:(half + 1) * 512],
                                         start=(pair == 0), stop=(pair == 7))

            # f gate (native Softplus) + beta (batched Sigmoid)
            f1b = p1s.tile([128, T], bf16, tag="f1b", name="f1b")
            wt1 = [p1s.tile([128, 128], bf16, tag=f"wf1_{k}", name=f"wf1_{k}")
                   for k in range(16)]
            for k in range(16):
                dma(wt1[k][:], wf1[k * 128:(k + 1) * 128, :])
            for half in range(2):
                ps = pj.tile([128, 512], fp32, tag="ps512", name="f1ps", bufs=2)
                for k in range(16):
                    nc.tensor.matmul(ps[:], wt1[k][:],
                                     ht[k][:, half * 512:(half + 1) * 512],
                                     start=(k == 0), stop=(k == 15))
                nc.scalar.copy(f1b[:, half * 512:(half + 1) * 512], ps[:])
            wf2t = p1s.tile([128, 32], bf16, tag="wf2t", name="wf2t")
            dma(wf2t[:], wf2[:])
            wbt = [p1s.tile([128, 4], bf16, tag=f"wb{k}", name=f"wbt{k}")
                   for k in range(16)]
            for k in range(16):
                dma(wbt[k][:], wb[k * 128:(k + 1) * 128, :])
            for half in range(2):
                spes = []
                for h in range(4):
                    gps = pj.tile([8, 512], fp32, tag="gps", name="gps",
                                  bufs=2)
                    nc.tensor.matmul(gps[:], wf2t[:, 8 * h:8 * h + 8],
                                     f1b[:, half * 512:(half + 1) * 512],
                                     start=True, stop=True)
                    spe = p1s.tile([8, 512], fp32, tag=f"spe{h % 2}",
                                   name=f"spe{h}", bufs=1)
                    nc.scalar.activation(spe[:], gps[:], Act.Exp,
                                         bias=dtbt[:, h:h + 1])
                    spes.append(spe)
                sps = []
                for h in range(4):
                    sp = p1s.tile([8, 512], fp32, tag="spx",
                                  name=f"sp{h}", bufs=2)
                    nc.scalar.activation(sp[:], spes[h][:], Act.Ln,
                                         bias=ones8[:, 0:1])
                    sps.append(sp)
                for h in range(4):
                    nc.vector.tensor_scalar(
                        gnah[h][:, half * 512:(half + 1) * 512], sps[h][:],
                        negat[:, h:h + 1], None, op0=Alu.mult)
            for half in range(2):
                bps = pj.tile([4, 512], fp32, tag="bps", name="bps")
                for k in range(16):
                    nc.tensor.matmul(bps[:], wbt[k][:],
                                     ht[k][:, half * 512:(half + 1) * 512],
                                     start=(k == 0), stop=(k == 15))
                nc.scalar.activation(bsg[:, half * 512:(half + 1) * 512],
                                     bps[:], Act.Sigmoid)

            wtq = load_w(0)

            for m in range(4):
                project(wtq, m, conv_slot=0, pair=m)
            wtk = load_w(1)
            for m in range(4):
                project(wtk, m, conv_slot=1, pair=4 + m)
            wtv = load_w(2)
            for m in range(4):
                project(wtv, m, dst_bf16=vb[m], conv_slot=2)
            wtg = load_w(3)
            for m in range(4):
                project(wtg, m, dst_bf16=gateb[m], gate_bias=bgt[:, m:m + 1])

            # l2 normalizers (batch Ln then Exp to avoid table thrash)
            recb = cons.tile([8, T], bf16, tag="recb", name="recb")
            nrmh = [p1s.tile([8, 512], fp32, tag=f"nrmh{i}", name=f"nrmh{i}")
                    for i in range(2)]
            for half in range(2):
                nc.scalar.activation(nrmh[half][:], ssqp[half][:], Act.Ln,
                                     scale=sc8t[:, 0:1], bias=eps8t[:, 0:1])
            for half in range(2):
                nc.scalar.activation(recb[:, half * 512:(half + 1) * 512],
                                     nrmh[half][:], Act.Exp, scale=-0.5)
            for pair in range(8):
                dst = qb[pair] if pair < 4 else kb[pair - 4]
                for half in range(2):
                    nb = pj.tile([128, 512], fp32, tag="ps512", name="nb",
                                 bufs=2)
                    nc.tensor.matmul(nb[:], s8b[:, pair * 128:(pair + 1) * 128],
                                     recb[:, half * 512:(half + 1) * 512],
                                     start=True, stop=True)
                    nc.vector.tensor_tensor(
                        dst[:, half * 512:(half + 1) * 512],
                        dst[:, half * 512:(half + 1) * 512],
                        nb[:], op=Alu.mult)

        # wo resident (DMA after phase-1 loads; used in phase 3)
        wot = [pers.tile([128, D], bf16, tag=f"wo{k}", name=f"wo{k}")
               for k in range(4)]
        for k in range(4):
            dma(wot[k][:], wo[k * 128:(k + 1) * 128, :])

        # =========== PHASE 2: chunked gated delta-rule recurrence ===========
        Sf = [st.tile([128, 128], fp32, tag=f"Sf{h}", name=f"Sf{h}")
              for h in range(4)]
        Sb = [st.tile([128, 128], bf16, tag=f"Sb{h}", name=f"Sb{h}")
              for h in range(4)]
        for h in range(4):
            nc.vector.memset(Sf[h][:], 0.0)
            nc.vector.memset(Sb[h][:], 0.0)
        with tc.tile_pool(name="pr", bufs=1, space="PSUM") as pr, \
             tc.tile_pool(name="p2s", bufs=2) as p2s:
            # PSUM rings (8 banks): pp x4 (bca/pall/red), xaq x1,
            # q32a x2 (prep/cfq/h2q/otq/suq), q16b x1 (htrq/ktq)
            def pp(nm):
                return pr.tile([128, 4 * C], fp32, tag="pp", bufs=4, name=nm)

            def q32(nm):
                return pr.tile([128, 4 * C], fp32, tag="q32a", bufs=2, name=nm)

            def q16(nm):
                return pr.tile([128, 4 * C], bf16, tag="q16b", bufs=1, name=nm)

            def stage_a_subs(ci):
                """State-independent work for chunk ci, as a drainable list
                of emission closures; the LAST element returns the ax dict
                when called (already-run closures are replaced by results).
                stage_b pops closures between solve levels."""
                state = {}
                subs = []

                def run_prep():
                    state.update(_a_prep(ci))

                def mk_eall(h):
                    return lambda: _a_eall(ci, h, state)

                def mk_corr(h, which):
                    return lambda: _a_corr(ci, h, which, state)

                subs.append(run_prep)
                for h in range(4):
                    subs.append(mk_eall(h))
                for h in range(4):
                    subs.append(mk_corr(h, 'M'))
                for h in range(4):
                    subs.append(mk_corr(h, 'G'))

                class Drain(list):
                    def __bool__(self2):
                        return True
                    def pop(self2):
                        while subs:
                            subs.pop(0)()
                        return state['ax']
                    def step(self2, n=1):
                        for _ in range(n):
                            if subs:
                                subs.pop(0)()
                return Drain()

            def stage_a(ci):
                d = stage_a_subs(ci)
                return d.pop()

            def _a_prep(ci):
                """State-independent prep for chunk ci."""
                ts = slice(ci * C, (ci + 1) * C)
                prep = q32(f"prep{ci}")
                nc.tensor.transpose(prep[:, 0:4], bsg[:, ts], idf[0:4, 0:4])
                beta2 = p2s.tile([128, 4], fp32, tag="beta2", name=f"beta2_{ci}")
                nc.scalar.copy(beta2[:], prep[:, 0:4])
                cNs, cNhi, cNlo = [], [], []
                for h in range(4):
                    cN = p2s.tile([8, C], fp32, tag=f"cN{h}", name=f"cN{h}_{ci}")
                    nc.vector.tensor_tensor_scan(cN[:], ones8[:],
                                                 gnah[h][:, ts], 0.0,
                                                 op0=Alu.mult, op1=Alu.add)
                    nc.tensor.transpose(prep[:, 4 + 8 * h:4 + 8 * h + 8],
                                        cN[:], idf[0:8, 0:8])
                    chi = p2s.tile([8, C], bf16, tag=f"cNhi{h}",
                                   name=f"cNhi{h}_{ci}")
                    nc.scalar.copy(chi[:], cN[:])
                    clo = p2s.tile([8, C], bf16, tag=f"cNlo{h}",
                                   name=f"cNlo{h}_{ci}")
                    nc.vector.tensor_tensor(clo[:], cN[:], chi[:],
                                            op=Alu.subtract)
                    cNs.append(cN); cNhi.append(chi); cNlo.append(clo)
                ncNt = p2s.tile([128, 32], fp32, tag="ncNt", name=f"ncNt{ci}")
                nc.scalar.copy(ncNt[:], prep[:, 4:36])

                # channel decay expansion, all heads in one quad
                cfq = q32(f"cfq{ci}")
                for h in range(4):
                    hs_ = slice(h * C, (h + 1) * C)
                    nc.tensor.matmul(cfq[:, hs_], replt[:], cNhi[h][:],
                                     start=True, stop=False)
                    nc.tensor.matmul(cfq[:, hs_], replt[:], cNlo[h][:],
                                     start=False, stop=True)
                nclq = p2s.tile([128, 4], fp32, tag="nclq", name=f"nclq{ci}")
                for h in range(4):
                    nc.vector.tensor_scalar(nclq[:, h:h + 1],
                                            cfq[:, h * C + C - 1:h * C + C],
                                            -1.0, None, op0=Alu.mult)
                bfq = p2s.tile([128, 4 * C], bf16, tag="bfq", name=f"bfq{ci}")
                nc.scalar.activation(bfq[:], cfq[:], Act.Exp, scale=-1.0)
                kfq = p2s.tile([128, 4 * C], bf16, tag="kfq", name=f"kfq{ci}")
                for h in range(4):
                    hs_ = slice(h * C, (h + 1) * C)
                    nc.scalar.activation(kfq[:, hs_], cfq[:, hs_], Act.Exp,
                                         bias=nclq[:, h:h + 1])
                bCq = p2s.tile([128, 4], fp32, tag="bCq", name=f"bCq{ci}")
                nc.scalar.activation(bCq[:], nclq[:], Act.Exp)
                nbfq = p2s.tile([128, 4 * C], bf16, tag="nbfq",
                                name=f"nbfq{ci}")
                nc.gpsimd.tensor_tensor(nbfq[:], bfq[:], neg4c[:],
                                        op=Alu.mult)

                # decayed k/q streams + mod-4 masked k
                negWt, qtT, kend, kmf = [], [], [], []
                for h in range(4):
                    hs_ = slice(h * C, (h + 1) * C)
                    nw = p2s.tile([128, C], bf16, tag=f"negWt{h}",
                                  name=f"negWt{h}_{ci}")
                    nc.gpsimd.tensor_tensor(nw[:], kb[h][:, ts],
                                            nbfq[:, hs_], op=Alu.mult)
                    qt = p2s.tile([128, C], bf16, tag=f"qtT{h}",
                                  name=f"qtT{h}_{ci}")
                    nc.gpsimd.tensor_tensor(qt[:], qb[h][:, ts], bfq[:, hs_],
                                            op=Alu.mult)
                    ke = p2s.tile([128, C], bf16, tag=f"kend{h}",
                                  name=f"kend{h}_{ci}")
                    nc.gpsimd.tensor_tensor(ke[:], kb[h][:, ts], kfq[:, hs_],
                                            op=Alu.mult)
                    kms = []
                    for j in range(4):
                        km = p2s.tile([128, C], bf16, tag=f"km{j}_{h}",
                                      name=f"km{j}_{h}_{ci}")
                        nc.vector.tensor_scalar(km[:], kb[h][:, ts],
                                                evodt[:, j:j + 1], None,
                                                op0=Alu.mult)
                        kms.append(km)
                    negWt.append(nw); qtT.append(qt); kend.append(ke)
                    kmf.append(kms)

                ealls = [None] * 4

                def corr(h, srcq, mask_t, scale_col, nm, dst):
                    prods = []
                    for half in range(2):
                        pall = pp(f"pall{nm}{h}_{half}_{ci}")
                        for j in range(4):
                            n = half * 4 + j
                            kmsk = kmf[h][n % 4]
                            blk = 64 * (n // 4)
                            nc.tensor.matmul(
                                pall[:, j * C:(j + 1) * C],
                                kmsk[blk:blk + 64, :],
                                srcq[blk:blk + 64, ts],
                                start=True, stop=True)
                        prod = p2s.tile([128, 4 * C], bf16, tag=f"prod{half}",
                                        name=f"prod{nm}{h}_{half}", bufs=2)
                        easl = ealls[h][:, half * 4 * C:(half + 1) * 4 * C]
                        nc.vector.scalar_tensor_tensor(prod[:], easl, 1.0,
                                                       pall[:], op0=Alu.min,
                                                       op1=Alu.mult)
                        prods.append(prod)
                    red = pp(f"red{nm}{h}_{ci}")
                    for n in range(NG):
                        nc.tensor.matmul(red[:, 0:C], idb[:],
                                         prods[n // 4][:, (n % 4) * C:
                                                       (n % 4 + 1) * C],
                                         start=(n == 0), stop=(n == NG - 1))
                    if scale_col is not None:
                        nc.vector.scalar_tensor_tensor(dst, red[:, 0:C],
                                                       scale_col, mask_t[:],
                                                       op0=Alu.mult,
                                                       op1=Alu.mult)
                    else:
                        nc.vector.tensor_tensor(dst, red[:, 0:C], mask_t[:],
                                                op=Alu.mult)

                Hq0 = p2s.tile([128, 4 * C], bf16, tag="Hq", name=f"Hq{ci}_0")
                Gq = p2s.tile([128, 4 * C], bf16, tag="Gq", name=f"Gq{ci}")
                ax = dict(ts=ts, beta2=beta2, bCq=bCq, negWt=negWt,
                          qtT=qtT, kend=kend, Hq0=Hq0, Gq=Gq, ci=ci,
                          ealls=ealls, kmf=kmf, corr=corr, cNhi=cNhi,
                          cNlo=cNlo, ncNt=ncNt)
                return dict(ax=ax)

            def _a_eall(ci, h, state):
                ax = state['ax']
                cNhi, cNlo, ncNt = ax['cNhi'], ax['cNlo'], ax['ncNt']
                ea = p2s.tile([128, NG * C], bf16, tag=f"eall{h}",
                              name=f"eall{h}_{ci}", bufs=2)
                for half in range(2):
                    bca = pp(f"bca{h}_{half}_{ci}")
                    for j in range(4):
                        n = half * 4 + j
                        nc.tensor.matmul(bca[:, j * C:(j + 1) * C],
                                         s8f[:, n * 128:(n + 1) * 128],
                                         cNhi[h][:], start=True, stop=False)
                        nc.tensor.matmul(bca[:, j * C:(j + 1) * C],
                                         s8f[:, n * 128:(n + 1) * 128],
                                         cNlo[h][:], start=False, stop=True)
                    for j in range(4):
                        n = half * 4 + j
                        nc.scalar.activation(
                            ea[:, n * C:(n + 1) * C],
                            bca[:, j * C:(j + 1) * C], Act.Exp,
                            scale=-1.0,
                            bias=ncNt[:, 8 * h + n:8 * h + n + 1])
                ax['ealls'][h] = ea

            def _a_corr(ci, h, which, state):
                ax = state['ax']
                if which == 'M':
                    ax['corr'](h, kb[h], nmM, ax['beta2'][:, h:h + 1], "M",
                               ax['Hq0'][:, h * C:(h + 1) * C])
                else:
                    ax['corr'](h, qb[h], mGt, None, "G",
                               ax['Gq'][:, h * C:(h + 1) * C])

            def stage_b(ax, sub_next=()):
                """State-dependent solve + output + state update,
                interleaved with the next chunk's independent work."""
                ci, ts = ax['ci'], ax['ts']
                beta2, bCq = ax['beta2'], ax['bCq']
                negWt, qtT, kend = ax['negWt'], ax['qtT'], ax['kend']
                Hq, Gq = ax['Hq0'], ax['Gq']

                xaq = pr.tile([128, 4 * C], fp32, tag="xaq", bufs=1,
                              name=f"xaq{ci}")
                xaccs = [xaq[:, h * C:(h + 1) * C] for h in range(4)]
                for h in range(4):
                    nc.tensor.matmul(xaccs[h], vb[h][:, ts], idb[:],
                                     start=True, stop=False)
                    nc.tensor.matmul(xaccs[h], negWt[h][:], Sb[h][:],
                                     start=False, stop=True)
                for lev in range(NLEV):
                    if sub_next:
                        sub_next.step(2)
                    last = (lev == NLEV - 1)
                    xbq = p2s.tile([128, 4 * C], bf16, tag="xbq",
                                   name=f"xbq{ci}_{lev}")
                    nc.scalar.copy(xbq[:], xaq[:])
                    xaq = pr.tile([128, 4 * C], fp32, tag="xaq", bufs=1,
                                  name=f"xaq{ci}_{lev}")
                    xaccs = [xaq[:, h * C:(h + 1) * C] for h in range(4)]
                    for h in range(4):
                        hs_ = slice(h * C, (h + 1) * C)
                        nc.tensor.matmul(xaccs[h], idb[:], xbq[:, hs_],
                                         start=True, stop=False)
                        nc.tensor.matmul(xaccs[h], Hq[:, hs_], xbq[:, hs_],
                                         start=False, stop=True)
                    if not last:
                        htrq = q16(f"htr{ci}_{lev}")
                        for h in range(4):
                            nc.tensor.transpose(htrq[:, h * C:(h + 1) * C],
                                                Hq[:, h * C:(h + 1) * C],
                                                idb[:])
                        htsq = p2s.tile([128, 4 * C], bf16, tag="htsq",
                                        name=f"htsq{ci}_{lev}")
                        nc.scalar.copy(htsq[:], htrq[:])
                        h2q = q32(f"h2q{ci}_{lev}")
                        for h in range(4):
                            hs_ = slice(h * C, (h + 1) * C)
                            nc.tensor.matmul(h2q[:, hs_], htsq[:, hs_],
                                             Hq[:, hs_], start=True, stop=True)
                        Hq = p2s.tile([128, 4 * C], bf16, tag="Hq",
                                      name=f"Hq{ci}_{lev + 1}")
                        nc.scalar.copy(Hq[:], h2q[:])

                ubs = []
                for h in range(4):
                    ub = p2s.tile([128, C], bf16, tag=f"ub{h}",
                                  name=f"ub{h}_{ci}")
                    nc.vector.tensor_scalar(ub[:], xaccs[h],
                                            beta2[:, h:h + 1], None,
                                            op0=Alu.mult)
                    ubs.append(ub)
                otq = q32(f"otq{ci}")
                ktq = q16(f"ktq{ci}")
                for h in range(4):
                    hs_ = slice(h * C, (h + 1) * C)
                    nc.tensor.matmul(otq[:, hs_], Sb[h][:], qtT[h][:],
                                     start=True, stop=False)
                    nc.tensor.matmul(otq[:, hs_], ubs[h][:],
                                     Gq[:, hs_], start=False, stop=True)
                    nc.tensor.transpose(ktq[:, hs_], kend[h][:], idb[:])
                ktsq = p2s.tile([128, 4 * C], bf16, tag="ktsq",
                                name=f"ktsq{ci}")
                nc.scalar.copy(ktsq[:], ktq[:])
                suq = q32(f"suq{ci}")
                for h in range(4):
                    hs_ = slice(h * C, (h + 1) * C)
                    nc.tensor.matmul(suq[:, hs_], ktsq[:, hs_], ubs[h][:],
                                     start=True, stop=True)
                    nc.vector.scalar_tensor_tensor(Sf[h][:], Sf[h][:],
                                                   bCq[:, h:h + 1],
                                                   suq[:, hs_],
                                                   op0=Alu.mult, op1=Alu.add)
                    nc.scalar.copy(Sb[h][:], Sf[h][:])
                sspq = None
                for h in range(4):
                    hs_ = slice(h * C, (h + 1) * C)
                    yf = yfall[h]
                    nc.vector.tensor_tensor(yf[:, ts], gateb[h][:, ts],
                                            otq[:, hs_], op=Alu.mult)
                    ysq = p2s.tile([128, C], bf16, tag=f"ysq{h}",
                                   name=f"ysq{h}_{ci}")
                    nc.gpsimd.tensor_tensor(ysq[:], yf[:, ts], yf[:, ts],
                                            op=Alu.mult)
                    if h == 0:
                        sspq = pr.tile([128, 4 * C], fp32, tag="xaq", bufs=1,
                                       name=f"ssp{ci}")
                    nc.tensor.matmul(sspq[0:4, 0:C], oh4t[:, 4 * h:4 * h + 4],
                                     ysq[:], start=(h == 0), stop=(h == 3))
                    if h == 3:
                        nc.scalar.copy(sqs[:, ts], sspq[0:4, 0:C])
                if sub_next:
                    sub_next.step(3)

            ax_prev = stage_a(0)
            for ci in range(NCH):
                sub_next = stage_a_subs(ci + 1) if ci + 1 < NCH else []
                stage_b(ax_prev, sub_next)
                ax_prev = sub_next.pop() if sub_next else None

        # =========== PHASE 3: gated RMS norm + output projection ===========
        with tc.tile_pool(name="po", bufs=1, space="PSUM") as po:
            nrm4 = chk.tile([4, T], fp32, tag="nrm4", name="nrm4")
            nc.scalar.activation(nrm4[:], sqs[:], Act.Ln, scale=1.0 / DV,
                                 bias=eps4[:, 0:1])
            rst4 = chk.tile([4, T], bf16, tag="rst4", name="rst4")
            nc.scalar.activation(rst4[:], nrm4[:], Act.Exp, scale=-0.5)
            for half in range(2):
                sl = slice(half * 512, (half + 1) * 512)
                for h in range(4):
                    rbc = po.tile([128, 512], fp32, tag="rbc", bufs=2,
                                  name=f"rbc{h}_{half}")
                    nc.tensor.matmul(rbc[:], oh4bt[:, h * 128:(h + 1) * 128],
                                     rst4[:, sl], start=True, stop=True)
                    nc.vector.scalar_tensor_tensor(yfall[h][:, sl],
                                                   yfall[h][:, sl],
                                                   nwt[:, 0:1], rbc[:],
                                                   op0=Alu.mult, op1=Alu.mult)
                for m in range(16):
                    ps = po.tile([128, 512], fp32, tag="ops", bufs=3,
                                 name="ops")
                    for k in range(4):
                        nc.tensor.matmul(ps[:], wot[k][:, m * 128:(m + 1) * 128],
                                         yfall[k][:, sl],
                                         start=(k == 0), stop=(k == 3))
                    osb = convp.tile([128, 512], fp32, tag="osb", name="osb",
                                     bufs=3)
                    if m % 2 == 0:
                        nc.vector.tensor_copy(osb[:], ps[:])
                    else:
                        nc.scalar.copy(osb[:], ps[:])
                    dma(outT[m * 128:(m + 1) * 128, sl], osb[:])

    nc.compile()
    return nc


def _prep_inputs(inputs):
    f32 = np.float32
    hs = np.asarray(inputs['hidden_states'], f32)
    maps = []
    tri = np.tril(np.ones((C, C), f32))
    nmaskM = (-(1.0 - tri)).astype(BF)                      # -1 strictly upper
    maskG = (1.0 - tri + np.eye(C, dtype=f32)).astype(BF)   # +1 upper incl diag
    repl = np.zeros((NG, DK), f32)
    for n in range(NG):
        repl[n, n * GG:(n + 1) * GG] = 1.0
    sel8 = np.zeros((NG, NG * 128), f32)
    for n in range(NG):
        sel8[n, n * 128:(n + 1) * 128] = 1.0
    oh8 = np.zeros((DK, 64), f32)
    for i in range(8):
        oh8[:, i * 8 + i] = 1.0
    oh4 = np.zeros((DK, 16), f32)
    for i in range(4):
        oh4[:, i * 4 + i] = 1.0
    oh4b = np.zeros((4, 4 * DK), f32)
    for i in range(4):
        oh4b[i, i * 128:(i + 1) * 128] = 1.0
    evod = np.zeros((DK, 4), f32)
    for cc in range(DK):
        evod[cc, (cc // GG) % 4] = 1.0
    ident = np.eye(128, dtype=f32)
    hTs = [np.ascontiguousarray(hs[b].T).astype(BF) for b in range(B)]
    for c in range(8):
        b, hg = c // 4, c % 4
        cols = slice(hg * NH * DK, (hg + 1) * NH * DK)
        gcols = slice(hg * NH * NG, (hg + 1) * NH * NG)
        hcols = slice(hg * NH, (hg + 1) * NH)
        nega = np.tile(np.exp(np.asarray(inputs['A_log'], f32)[hcols])[None, :],
                       (NG, 1))
        m = {
            'hT': hTs[b],
            'wqkvg': np.ascontiguousarray(np.concatenate(
                [np.asarray(inputs['Wq'], f32)[:, cols],
                 np.asarray(inputs['Wk'], f32)[:, cols],
                 np.asarray(inputs['Wv'], f32)[:, cols],
                 np.asarray(inputs['Wg'], f32)[:, cols]], 1)).astype(BF),
            'wo': np.asarray(inputs['Wo'], f32)[cols, :].astype(BF),
            'wf1': np.asarray(inputs['Wf1'], f32).astype(BF),
            'wf2': np.asarray(inputs['Wf2'], f32)[:, gcols].astype(BF),
            'wb': np.asarray(inputs['Wb'], f32)[:, hcols].astype(BF),
            'cw': np.ascontiguousarray(np.concatenate(
                [np.asarray(inputs['conv_q'], f32)[cols],
                 np.asarray(inputs['conv_k'], f32)[cols],
                 np.asarray(inputs['conv_v'], f32)[cols]], 1)),
            'nega': np.ascontiguousarray(nega).astype(f32),
            'dtb': np.ascontiguousarray(
                np.asarray(inputs['dt_bias'], f32)[gcols].reshape(NH, NG).T),
            'bgc': np.ascontiguousarray(
                np.asarray(inputs['bg'], f32)[cols].reshape(NH, DV).T),
            'normw': np.ascontiguousarray(
                np.asarray(inputs['norm_w'], f32)[:, None]),
            'repl': repl.astype(BF),
            'self8f': sel8.astype(BF),
            'sel8b': sel8.astype(BF),
            'onescol': np.ones((DK, 1), f32).astype(BF),
            'oh4': oh4.astype(BF),
            'oh4b': oh4b.astype(BF),
            'evod': evod,
            'oh8': oh8.astype(BF),
            'sc8': np.array([[1.0 / SCALE ** 2]] * 4 + [[1.0]] * 4, f32),
            'eps8': np.array([[1e-6 / SCALE ** 2]] * 4 + [[1e-6]] * 4, f32),
            'nmaskM': nmaskM,
            'maskG': maskG,
            'idbf': ident.astype(BF),
            'id4f': np.tile(ident, (1, 4)).astype(BF),
            'idf32': ident,
        }
        maps.append(m)
    return maps


def kernel(**inputs):
    from concourse.bass_utils import run_bass_kernel_spmd
    if 'nc' not in _CACHE:
        _CACHE['nc'] = _build()
    nc = _CACHE['nc']
    maps = _prep_inputs(inputs)
    res = run_bass_kernel_spmd(nc, maps, list(range(8))).results
    out = np.zeros((B, T, D), np.float32)
    for c in range(8):
        out[c // 4] += res[c]['outT'].T.astype(np.float32)
    return out


# revision 33
# speedup vs baseline: 1.0523x; 1.0523x over previous
"""Grouped gated DeltaNet (KDA-style) on 8 TRN2 NeuronCores — v2.

Sharding: core c -> (batch b = c//4, head-group hg = c%4 of 4 heads).

v2 restructure vs baseline:
- no act-table thrash: recurrence loop uses only Exp/Copy/Square (one set);
  RMS-norm Ln/Exp deferred to a single post-loop pass
- group k/q correlations via partition-sliced matmuls (no masked-k copies)
- decay-difference exp: PE broadcast (fp32) + 8 bias-fused Act exps; clamp
  fused into the weighting multiply (min(1,e)*pall)
- group-weighted reduction on PE (one-hot accumulating matmuls)
- triangular solve: Neumann doubling (5 levels), closed PSUM groups
- small elementwise work offloaded to the GpSimd engine
- quad (4-head) tiles for wide low-overhead ops
- cross-chunk software pipelining: next chunk's state-independent work
  interleaved into the current chunk's solve

Self-contained: B=2, T=1024, D=2048, H=16, DK=DV=128 hardcoded.
"""
import sys
sys.path.insert(0, '/opt/trn_rl_repo')
import numpy as np
import ml_dtypes
from contextlib import ExitStack

B, T, D = 2, 1024, 2048
H, DK, DV, GG = 16, 128, 128, 16
NG = DK // GG          # 8 gate groups per head
NH = 4                 # heads per core
C = 128                # chunk length
NCH = T // C
NLEV = 4               # Neumann doubling levels (covers N^k, k < 2^NLEV)
SCALE = DK ** -0.5
EPS = 1e-5

BF = ml_dtypes.bfloat16
_CACHE = {}


def _build():
    import concourse.tile as tile
    from concourse import bacc, mybir

    fp32 = mybir.dt.float32
    bf16 = mybir.dt.bfloat16
    Alu = mybir.AluOpType
    Act = mybir.ActivationFunctionType

    nc = bacc.Bacc("TRN2", target_bir_lowering=False, debug=False, num_devices=8)
    dp = lambda n, sh, dt: nc.dram_tensor(n, sh, dt, kind="ExternalInput").ap()
    hT = dp("hT", [D, T], bf16)
    wqkvg = dp("wqkvg", [D, 4 * NH * DK], bf16)
    wo = dp("wo", [NH * DV, D], bf16)
    wf1 = dp("wf1", [D, DV], bf16)
    wf2 = dp("wf2", [DV, NH * NG], bf16)
    wb = dp("wb", [D, NH], bf16)
    cw = dp("cw", [NH * DK, 12], fp32)
    nega = dp("nega", [NG, NH], fp32)
    dtb = dp("dtb", [NG, NH], fp32)
    bgc = dp("bgc", [DV, NH], fp32)
    normw = dp("normw", [DV, 1], fp32)
    repl = dp("repl", [NG, DK], bf16)
    self8f = dp("self8f", [NG, NG * C], bf16)
    onescol = dp("onescol", [DK, 1], bf16)
    oh4 = dp("oh4", [DK, 16], bf16)
    oh4b = dp("oh4b", [4, 4 * DK], bf16)
    evod = dp("evod", [DK, 4], fp32)
    oh8 = dp("oh8", [DK, 64], bf16)
    sel8b = dp("sel8b", [8, 8 * 128], bf16)
    sc8 = dp("sc8", [8, 1], fp32)
    eps8 = dp("eps8", [8, 1], fp32)
    nmaskM = dp("nmaskM", [C, C], bf16)   # -1 strictly upper (s<t)
    maskG = dp("maskG", [C, C], bf16)     # +1 upper incl diag (s<=t)
    idbf = dp("idbf", [128, 128], bf16)
    id4f = dp("id4f", [128, 4 * 128], bf16)
    idf32 = dp("idf32", [128, 128], fp32)
    outT = nc.dram_tensor("outT", [D, T], fp32, kind="ExternalOutput").ap()
    DBG = bool(__import__('os').environ.get('K2_DEBUG'))
    if DBG:
        dbg = {n: nc.dram_tensor(n, [128, T], fp32, kind="ExternalOutput").ap()
               for n in ['d_qb', 'd_kb', 'd_vb', 'd_gateb', 'd_yf']}
        dbg['d_gna'] = nc.dram_tensor('d_gna', [8, T], fp32,
                                      kind="ExternalOutput").ap()
        dbg['d_bsg'] = nc.dram_tensor('d_bsg', [4, T], fp32,
                                      kind="ExternalOutput").ap()
        dbg['d_sqs'] = nc.dram_tensor('d_sqs', [4, T], fp32,
                                      kind="ExternalOutput").ap()
        dbg['d_eall'] = nc.dram_tensor('d_eall', [128, NG * C], bf16,
                                       kind="ExternalOutput").ap()
        for n in ['d_H0', 'd_GtM', 'd_ub', 'd_cN']:
            dbg[n] = nc.dram_tensor(n, [128, C], fp32,
                                    kind="ExternalOutput").ap()

    with tile.TileContext(nc) as tc, ExitStack() as ctx:
        pool = lambda name, bufs, space="SBUF": ctx.enter_context(
            tc.tile_pool(name=name, bufs=bufs, space=space))

        cons = pool("cons", 1)
        htp = pool("htp", 1)
        wst = pool("wst", 1)
        wsm = pool("wsm", 1)
        pers = pool("pers", 1)
        convp = pool("convp", 1)
        chk = pool("chk", 2)
        st = pool("st", 1)

        dma = nc.sync.dma_start

        # ---- consts ----
        cwt = []
        for m in range(4):
            t = cons.tile([128, 12], fp32, tag=f"cw{m}", name=f"cw{m}")
            dma(t[:], cw[m * 128:(m + 1) * 128, :])
            cwt.append(t)

        def ctile(shape, dt, src, nm):
            t = cons.tile(shape, dt, tag=nm, name=nm)
            dma(t[:], src[:])
            return t
        negat = ctile([8, 4], fp32, nega, "negat")
        dtbt = ctile([8, 4], fp32, dtb, "dtbt")
        bgt = ctile([128, 4], fp32, bgc, "bgt")
        nwt = ctile([128, 1], fp32, normw, "nwt")
        replt = ctile([8, 128], bf16, repl, "replt")
        s8f = ctile([NG, NG * C], bf16, self8f, "s8f")
        oct_ = ctile([128, 1], bf16, onescol, "oct")
        oh4t = ctile([128, 16], bf16, oh4, "oh4t")
        oh4bt = ctile([4, 4 * 128], bf16, oh4b, "oh4bt")
        evodt = ctile([128, 4], fp32, evod, "evodt")
        oh8t = ctile([128, 64], bf16, oh8, "oh8t")
        s8b = ctile([8, 8 * 128], bf16, sel8b, "s8b")
        sc8t = ctile([8, 1], fp32, sc8, "sc8t")
        eps8t = ctile([8, 1], fp32, eps8, "eps8t")
        nmM = ctile([128, 128], bf16, nmaskM, "nmM")
        mGt = ctile([128, 128], bf16, maskG, "mGt")
        idb = ctile([128, 128], bf16, idbf, "idb")
        id4 = ctile([128, 4 * 128], bf16, id4f, "id4")
        idf = ctile([128, 128], fp32, idf32, "idf")
        ones8 = cons.tile([8, C], fp32, tag="ones8", name="ones8")
        nc.vector.memset(ones8[:], 1.0)
        eps4 = cons.tile([4, 1], fp32, tag="eps4", name="eps4")
        nc.vector.memset(eps4[:], EPS)
        neg4c = cons.tile([128, 4 * C], bf16, tag="neg4c", name="neg4c")
        nc.vector.memset(neg4c[:], -1.0)

        # ---- persistent activations ----
        mk = lambda p, nm, dt=bf16, sh=None: [
            p.tile(sh or [128, T], dt, tag=f"{nm}{m}", name=f"{nm}{m}")
            for m in range(4)]
        qb, kb, vb = mk(pers, "qb"), mk(pers, "kb"), mk(pers, "vb")
        gateb, yfall = mk(pers, "gateb"), mk(pers, "yfall")
        gnah = [cons.tile([8, T], fp32, tag=f"gna{h}", name=f"gna{h}")
                for h in range(4)]
        bsg = cons.tile([4, T], fp32, tag="bsg", name="bsg")
        sqs = cons.tile([4, T], fp32, tag="sqs", name="sqs")

        # =========== PHASE 1: projections ===========
        with tc.tile_pool(name="pj", bufs=1, space="PSUM") as pj, \
             tc.tile_pool(name="p1s", bufs=1) as p1s:
            ht = []
            for k in range(16):
                t = p1s.tile([128, T], bf16, tag=f"ht{k}", name=f"ht{k}")
                dma(t[:], hT[k * 128:(k + 1) * 128, :])
                ht.append(t)
            ssqp = [pj.tile([8, 512], fp32, tag=f"ssq{hf}", name=f"ssq{hf}")
                    for hf in range(2)]

            def load_w(proj):
                wt = [p1s.tile([128, 512], bf16, tag=f"w{k}", name=f"wt{k}")
                      for k in range(16)]
                for k in range(16):
                    dma(wt[k][:], wqkvg[k * 128:(k + 1) * 128,
                                        proj * 512:(proj + 1) * 512])
                return wt

            def project(wt, m, dst_bf16=None, conv_slot=None, pair=None,
                        gate_bias=None):
                xpad = None
                if conv_slot is not None:
                    xpad = p1s.tile([128, T + 3], fp32, tag="xpad",
                                      name="xpad", bufs=2)
                    nc.vector.memset(xpad[:, 0:3], 0.0)
                for half in range(2):
                    ps = pj.tile([128, 512], fp32, tag="ps512", name="projps",
                                 bufs=2)
                    for k in range(16):
                        nc.tensor.matmul(ps[:], wt[k][:, m * 128:(m + 1) * 128],
                                         ht[k][:, half * 512:(half + 1) * 512],
                                         start=(k == 0), stop=(k == 15))
                    if xpad is not None:
                        nc.scalar.copy(xpad[:, 3 + half * 512:
                                            3 + (half + 1) * 512], ps[:])
                    elif gate_bias is not None:
                        nc.scalar.activation(
                            dst_bf16[:, half * 512:(half + 1) * 512],
                            ps[:], Act.Silu, bias=gate_bias)
                    else:
                        nc.scalar.copy(dst_bf16[:, half * 512:(half + 1) * 512],
                                       ps[:])
                if xpad is None:
                    return
                # depthwise causal conv on GpSimd (Pool) engine
                cwm = cwt[m]
                s = conv_slot * 4
                a = p1s.tile([128, T], fp32, tag="acca", name="acca")
                bt = p1s.tile([128, T], fp32, tag="accb", name="accb")
                nc.vector.tensor_scalar(a[:], xpad[:, 3:3 + T],
                                        cwm[:, s + 3:s + 4], None, op0=Alu.mult)
                cur, nxt = a, bt
                for kk in (2, 1, 0):
                    nc.vector.scalar_tensor_tensor(
                        nxt[:], xpad[:, kk:kk + T], cwm[:, s + kk:s + kk + 1],
                        cur[:], op0=Alu.mult, op1=Alu.add)
                    cur, nxt = nxt, cur
                if pair is None:
                    nc.scalar.activation(dst_bf16[:], cur[:], Act.Silu)
                else:
                    qsil = qb[pair] if pair < 4 else kb[pair - 4]
                    nc.scalar.activation(qsil[:], cur[:], Act.Silu)
                    sq = p1s.tile([128, T], bf16, tag="sq", name="sq", bufs=2)
                    nc.vector.tensor_tensor(sq[:], qsil[:], qsil[:], op=Alu.mult)
                    for half in range(2):
                        nc.tensor.matmul(ssqp[half][:],
                                         oh8t[:, pair * 8:pair * 8 + 8],
                                         sq[:, half * 512:(half + 1) * 512],
                                         start=(pair == 0), stop=(pair == 7))

            # f gate (native Softplus) + beta (batched Sigmoid)
            f1b = p1s.tile([128, T], bf16, tag="f1b", name="f1b")
            wt1 = [p1s.tile([128, 128], bf16, tag=f"wf1_{k}", name=f"wf1_{k}")
                   for k in range(16)]
            for k in range(16):
                dma(wt1[k][:], wf1[k * 128:(k + 1) * 128, :])
            for half in range(2):
                ps = pj.tile([128, 512], fp32, tag="ps512", name="f1ps", bufs=2)
                for k in range(16):
                    nc.tensor.matmul(ps[:], wt1[k][:],
                                     ht[k][:, half * 512:(half + 1) * 512],
                                     start=(k == 0), stop=(k == 15))
                nc.scalar.copy(f1b[:, half * 512:(half + 1) * 512], ps[:])
            wf2t = p1s.tile([128, 32], bf16, tag="wf2t", name="wf2t")
            dma(wf2t[:], wf2[:])
            wbt = [p1s.tile([128, 4], bf16, tag=f"wb{k}", name=f"wbt{k}")
                   for k in range(16)]
            for k in range(16):
                dma(wbt[k][:], wb[k * 128:(k + 1) * 128, :])
            for half in range(2):
                spes = []
                for h in range(4):
                    gps = pj.tile([8, 512], fp32, tag="gps", name="gps",
                                  bufs=2)
                    nc.tensor.matmul(gps[:], wf2t[:, 8 * h:8 * h + 8],
                                     f1b[:, half * 512:(half + 1) * 512],
                                     start=True, stop=True)
                    spe = p1s.tile([8, 512], fp32, tag=f"spe{h % 2}",
                                   name=f"spe{h}", bufs=1)
                    nc.scalar.activation(spe[:], gps[:], Act.Exp,
                                         bias=dtbt[:, h:h + 1])
                    spes.append(spe)
                sps = []
                for h in range(4):
                    sp = p1s.tile([8, 512], fp32, tag="spx",
                                  name=f"sp{h}", bufs=2)
                    nc.scalar.activation(sp[:], spes[h][:], Act.Ln,
                                         bias=ones8[:, 0:1])
                    sps.append(sp)
                for h in range(4):
                    nc.vector.tensor_scalar(
                        gnah[h][:, half * 512:(half + 1) * 512], sps[h][:],
                        negat[:, h:h + 1], None, op0=Alu.mult)
            for half in range(2):
                bps = pj.tile([4, 512], fp32, tag="bps", name="bps")
                for k in range(16):
                    nc.tensor.matmul(bps[:], wbt[k][:],
                                     ht[k][:, half * 512:(half + 1) * 512],
                                     start=(k == 0), stop=(k == 15))
                nc.scalar.activation(bsg[:, half * 512:(half + 1) * 512],
                                     bps[:], Act.Sigmoid)

            wtq = load_w(0)

            for m in range(4):
                project(wtq, m, conv_slot=0, pair=m)
            wtk = load_w(1)
            for m in range(4):
                project(wtk, m, conv_slot=1, pair=4 + m)
            wtv = load_w(2)
            for m in range(4):
                project(wtv, m, dst_bf16=vb[m], conv_slot=2)
            wtg = load_w(3)
            for m in range(4):
                project(wtg, m, dst_bf16=gateb[m], gate_bias=bgt[:, m:m + 1])

            # l2 normalizers (batch Ln then Exp to avoid table thrash)
            recb = cons.tile([8, T], bf16, tag="recb", name="recb")
            nrmh = [p1s.tile([8, 512], fp32, tag=f"nrmh{i}", name=f"nrmh{i}")
                    for i in range(2)]
            for half in range(2):
                nc.scalar.activation(nrmh[half][:], ssqp[half][:], Act.Ln,
                                     scale=sc8t[:, 0:1], bias=eps8t[:, 0:1])
            for half in range(2):
                nc.scalar.activation(recb[:, half * 512:(half + 1) * 512],
                                     nrmh[half][:], Act.Exp, scale=-0.5)
            for pair in range(8):
                dst = qb[pair] if pair < 4 else kb[pair - 4]
                for half in range(2):
                    nb = pj.tile([128, 512], fp32, tag="ps512", name="nb",
                                 bufs=2)
                    nc.tensor.matmul(nb[:], s8b[:, pair * 128:(pair + 1) * 128],
                                     recb[:, half * 512:(half + 1) * 512],
                                     start=True, stop=True)
                    nc.vector.tensor_tensor(
                        dst[:, half * 512:(half + 1) * 512],
                        dst[:, half * 512:(half + 1) * 512],
                        nb[:], op=Alu.mult)

        # wo resident (DMA after phase-1 loads; used in phase 3)
        wot = [pers.tile([128, D], bf16, tag=f"wo{k}", name=f"wo{k}")
               for k in range(4)]
        for k in range(4):
            dma(wot[k][:], wo[k * 128:(k + 1) * 128, :])

        # =========== PHASE 2: chunked gated delta-rule recurrence ===========
        Sf = [st.tile([128, 128], fp32, tag=f"Sf{h}", name=f"Sf{h}")
              for h in range(4)]
        Sb = [st.tile([128, 128], bf16, tag=f"Sb{h}", name=f"Sb{h}")
              for h in range(4)]
        for h in range(4):
            nc.vector.memset(Sf[h][:], 0.0)
            nc.vector.memset(Sb[h][:], 0.0)
        with tc.tile_pool(name="pr", bufs=1, space="PSUM") as pr, \
             tc.tile_pool(name="p2s", bufs=2) as p2s:
            # PSUM rings (8 banks): pp x4 (bca/pall/red), xaq x1,
            # q32a x2 (prep/cfq/h2q/otq/suq), q16b x1 (htrq/ktq)
            def pp(nm):
                return pr.tile([128, 4 * C], fp32, tag="pp", bufs=4, name=nm)

            def q32(nm):
                return pr.tile([128, 4 * C], fp32, tag="q32a", bufs=2, name=nm)

            def q16(nm):
                return pr.tile([128, 4 * C], bf16, tag="q16b", bufs=1, name=nm)

            def stage_a_subs(ci):
                """State-independent work for chunk ci, as a drainable list
                of emission closures; the LAST element returns the ax dict
                when called (already-run closures are replaced by results).
                stage_b pops closures between solve levels."""
                state = {}
                subs = []

                def run_prep():
                    state.update(_a_prep(ci))

                def mk_eall(h):
                    return lambda: _a_eall(ci, h, state)

                def mk_corr(h, which):
                    return lambda: _a_corr(ci, h, which, state)

                subs.append(run_prep)
                for h in range(4):
                    subs.append(mk_eall(h))
                for h in range(4):
                    subs.append(mk_corr(h, 'M'))
                for h in range(4):
                    subs.append(mk_corr(h, 'G'))

                class Drain(list):
                    def __bool__(self2):
                        return True
                    def pop(self2):
                        while subs:
                            subs.pop(0)()
                        return state['ax']
                    def step(self2, n=1):
                        for _ in range(n):
                            if subs:
                                subs.pop(0)()
                return Drain()

            def stage_a(ci):
                d = stage_a_subs(ci)
                return d.pop()

            def _a_prep(ci):
                """State-independent prep for chunk ci."""
                ts = slice(ci * C, (ci + 1) * C)
                prep = q32(f"prep{ci}")
                nc.tensor.transpose(prep[:, 0:4], bsg[:, ts], idf[0:4, 0:4])
                beta2 = p2s.tile([128, 4], fp32, tag="beta2", name=f"beta2_{ci}")
                nc.scalar.copy(beta2[:], prep[:, 0:4])
                cNs, cNhi, cNlo, ncNhi, ncNlo = [], [], [], [], []
                for h in range(4):
                    cN = p2s.tile([8, C], fp32, tag=f"cN{h}", name=f"cN{h}_{ci}")
                    nc.vector.tensor_tensor_scan(cN[:], ones8[:],
                                                 gnah[h][:, ts], 0.0,
                                                 op0=Alu.mult, op1=Alu.add)
                    nc.tensor.transpose(prep[:, 4 + 8 * h:4 + 8 * h + 8],
                                        cN[:], idf[0:8, 0:8])
                    chi = p2s.tile([8, C], bf16, tag=f"cNhi{h}",
                                   name=f"cNhi{h}_{ci}")
                    nc.scalar.copy(chi[:], cN[:])
                    clo = p2s.tile([8, C], bf16, tag=f"cNlo{h}",
                                   name=f"cNlo{h}_{ci}")
                    nc.vector.tensor_tensor(clo[:], cN[:], chi[:],
                                            op=Alu.subtract)
                    nhi = p2s.tile([8, C], bf16, tag=f"ncNhi{h}",
                                   name=f"ncNhi{h}_{ci}")
                    nc.vector.tensor_scalar(nhi[:], chi[:], -1.0, None,
                                            op0=Alu.mult)
                    nlo = p2s.tile([8, C], bf16, tag=f"ncNlo{h}",
                                   name=f"ncNlo{h}_{ci}")
                    nc.vector.tensor_scalar(nlo[:], clo[:], -1.0, None,
                                            op0=Alu.mult)
                    cNs.append(cN); cNhi.append(chi); cNlo.append(clo)
                    ncNhi.append(nhi); ncNlo.append(nlo)
                ncNt = p2s.tile([128, 32], fp32, tag="ncNt", name=f"ncNt{ci}")
                nc.scalar.copy(ncNt[:], prep[:, 4:36])

                # channel decay expansion, all heads in one quad
                cfq = q32(f"cfq{ci}")
                for h in range(4):
                    hs_ = slice(h * C, (h + 1) * C)
                    nc.tensor.matmul(cfq[:, hs_], replt[:], cNhi[h][:],
                                     start=True, stop=False)
                    nc.tensor.matmul(cfq[:, hs_], replt[:], cNlo[h][:],
                                     start=False, stop=True)
                nclq = p2s.tile([128, 4], fp32, tag="nclq", name=f"nclq{ci}")
                for h in range(4):
                    nc.vector.tensor_scalar(nclq[:, h:h + 1],
                                            cfq[:, h * C + C - 1:h * C + C],
                                            -1.0, None, op0=Alu.mult)
                bfq = p2s.tile([128, 4 * C], bf16, tag="bfq", name=f"bfq{ci}")
                nc.scalar.activation(bfq[:], cfq[:], Act.Exp, scale=-1.0)
                kfq = p2s.tile([128, 4 * C], bf16, tag="kfq", name=f"kfq{ci}")
                for h in range(4):
                    hs_ = slice(h * C, (h + 1) * C)
                    nc.scalar.activation(kfq[:, hs_], cfq[:, hs_], Act.Exp,
                                         bias=nclq[:, h:h + 1])
                bCq = p2s.tile([128, 4], fp32, tag="bCq", name=f"bCq{ci}")
                nc.scalar.activation(bCq[:], nclq[:], Act.Exp)
                nbfq = p2s.tile([128, 4 * C], bf16, tag="nbfq",
                                name=f"nbfq{ci}")
                nc.gpsimd.tensor_tensor(nbfq[:], bfq[:], neg4c[:],
                                        op=Alu.mult)

                # decayed k/q streams + mod-4 masked k
                negWt, qtT, kend, kmf = [], [], [], []
                for h in range(4):
                    hs_ = slice(h * C, (h + 1) * C)
                    nw = p2s.tile([128, C], bf16, tag=f"negWt{h}",
                                  name=f"negWt{h}_{ci}")
                    nc.gpsimd.tensor_tensor(nw[:], kb[h][:, ts],
                                            nbfq[:, hs_], op=Alu.mult)
                    qt = p2s.tile([128, C], bf16, tag=f"qtT{h}",
                                  name=f"qtT{h}_{ci}")
                    nc.gpsimd.tensor_tensor(qt[:], qb[h][:, ts], bfq[:, hs_],
                                            op=Alu.mult)
                    ke = p2s.tile([128, C], bf16, tag=f"kend{h}",
                                  name=f"kend{h}_{ci}")
                    nc.gpsimd.tensor_tensor(ke[:], kb[h][:, ts], kfq[:, hs_],
                                            op=Alu.mult)
                    kms = []
                    for j in range(4):
                        km = p2s.tile([128, C], bf16, tag=f"km{j}_{h}",
                                      name=f"km{j}_{h}_{ci}")
                        nc.vector.tensor_scalar(km[:], kb[h][:, ts],
                                                evodt[:, j:j + 1], None,
                                                op0=Alu.mult)
                        kms.append(km)
                    negWt.append(nw); qtT.append(qt); kend.append(ke)
                    kmf.append(kms)

                ealls = [None] * 4

                def corr(h, srcq, mask_t, scale_col, nm, dst):
                    prods = []
                    for half in range(2):
                        pall = pp(f"pall{nm}{h}_{half}_{ci}")
                        for j in range(4):
                            n = half * 4 + j
                            kmsk = kmf[h][n % 4]
                            blk = 64 * (n // 4)
                            nc.tensor.matmul(
                                pall[:, j * C:(j + 1) * C],
                                kmsk[blk:blk + 64, :],
                                srcq[blk:blk + 64, ts],
                                start=True, stop=True)
                        prod = p2s.tile([128, 4 * C], bf16, tag=f"prod{half}",
                                        name=f"prod{nm}{h}_{half}", bufs=2)
                        easl = ealls[h][:, half * 4 * C:(half + 1) * 4 * C]
                        nc.vector.scalar_tensor_tensor(prod[:], easl, 1.0,
                                                       pall[:], op0=Alu.min,
                                                       op1=Alu.mult)
                        prods.append(prod)
                    red = pp(f"red{nm}{h}_{ci}")
                    for n in range(NG):
                        nc.tensor.matmul(red[:, 0:C], idb[:],
                                         prods[n // 4][:, (n % 4) * C:
                                                       (n % 4 + 1) * C],
                                         start=(n == 0), stop=(n == NG - 1))
                    if scale_col is not None:
                        nc.vector.scalar_tensor_tensor(dst, red[:, 0:C],
                                                       scale_col, mask_t[:],
                                                       op0=Alu.mult,
                                                       op1=Alu.mult)
                    else:
                        nc.vector.tensor_tensor(dst, red[:, 0:C], mask_t[:],
                                                op=Alu.mult)

                Hq0 = p2s.tile([128, 4 * C], bf16, tag="Hq", name=f"Hq{ci}_0")
                Gq = p2s.tile([128, 4 * C], bf16, tag="Gq", name=f"Gq{ci}")
                ax = dict(ts=ts, beta2=beta2, bCq=bCq, negWt=negWt,
                          qtT=qtT, kend=kend, Hq0=Hq0, Gq=Gq, ci=ci,
                          ealls=ealls, kmf=kmf, corr=corr, cNhi=cNhi,
                          cNlo=cNlo, ncNt=ncNt, ncNhi=ncNhi, ncNlo=ncNlo)
                return dict(ax=ax)

            def _a_eall(ci, h, state):
                ax = state['ax']
                cNhi, cNlo, ncNt = ax['cNhi'], ax['cNlo'], ax['ncNt']
                ea = p2s.tile([128, NG * C], bf16, tag=f"eall{h}",
                              name=f"eall{h}_{ci}", bufs=2)
                ncNhi, ncNlo = ax['ncNhi'], ax['ncNlo']
                for half in range(2):
                    bca = pp(f"bca{h}_{half}_{ci}")
                    sl8 = s8f[:, half * 4 * 128:(half + 1) * 4 * 128]
                    nc.tensor.matmul(bca[:], ncNhi[h][:], sl8,
                                     start=True, stop=False)
                    nc.tensor.matmul(bca[:], ncNlo[h][:], sl8,
                                     start=False, stop=False)
                    for j in range(4):
                        n = half * 4 + j
                        nc.tensor.matmul(bca[:, j * C:(j + 1) * C],
                                         s8f[:, n * 128:(n + 1) * 128],
                                         cNhi[h][:], start=False, stop=False)
                        nc.tensor.matmul(bca[:, j * C:(j + 1) * C],
                                         s8f[:, n * 128:(n + 1) * 128],
                                         cNlo[h][:], start=False, stop=True)
                    nc.scalar.activation(
                        ea[:, half * 4 * C:(half + 1) * 4 * C], bca[:],
                        Act.Exp, scale=-1.0)
                ax['ealls'][h] = ea

            def _a_corr(ci, h, which, state):
                ax = state['ax']
                if which == 'M':
                    ax['corr'](h, kb[h], nmM, ax['beta2'][:, h:h + 1], "M",
                               ax['Hq0'][:, h * C:(h + 1) * C])
                else:
                    ax['corr'](h, qb[h], mGt, None, "G",
                               ax['Gq'][:, h * C:(h + 1) * C])

            def stage_b(ax, sub_next=()):
                """State-dependent solve + output + state update,
                interleaved with the next chunk's independent work."""
                ci, ts = ax['ci'], ax['ts']
                beta2, bCq = ax['beta2'], ax['bCq']
                negWt, qtT, kend = ax['negWt'], ax['qtT'], ax['kend']
                Hq, Gq = ax['Hq0'], ax['Gq']

                xaq = pr.tile([128, 4 * C], fp32, tag="xaq", bufs=1,
                              name=f"xaq{ci}")
                xaccs = [xaq[:, h * C:(h + 1) * C] for h in range(4)]
                for h in range(4):
                    nc.tensor.matmul(xaccs[h], vb[h][:, ts], idb[:],
                                     start=True, stop=False)
                    nc.tensor.matmul(xaccs[h], negWt[h][:], Sb[h][:],
                                     start=False, stop=True)
                for lev in range(NLEV):
                    if sub_next:
                        sub_next.step(2)
                    last = (lev == NLEV - 1)
                    xbq = p2s.tile([128, 4 * C], bf16, tag="xbq",
                                   name=f"xbq{ci}_{lev}")
                    nc.scalar.copy(xbq[:], xaq[:])
                    xaq = pr.tile([128, 4 * C], fp32, tag="xaq", bufs=1,
                                  name=f"xaq{ci}_{lev}")
                    xaccs = [xaq[:, h * C:(h + 1) * C] for h in range(4)]
                    for h in range(4):
                        hs_ = slice(h * C, (h + 1) * C)
                        nc.tensor.matmul(xaccs[h], idb[:], xbq[:, hs_],
                                         start=True, stop=False)
                        nc.tensor.matmul(xaccs[h], Hq[:, hs_], xbq[:, hs_],
                                         start=False, stop=True)
                    if not last:
                        htrq = q16(f"htr{ci}_{lev}")
                        for h in range(4):
                            nc.tensor.transpose(htrq[:, h * C:(h + 1) * C],
                                                Hq[:, h * C:(h + 1) * C],
                                                idb[:])
                        htsq = p2s.tile([128, 4 * C], bf16, tag="htsq",
                                        name=f"htsq{ci}_{lev}")
                        nc.scalar.copy(htsq[:], htrq[:])
                        h2q = q32(f"h2q{ci}_{lev}")
                        for h in range(4):
                            hs_ = slice(h * C, (h + 1) * C)
                            nc.tensor.matmul(h2q[:, hs_], htsq[:, hs_],
                                             Hq[:, hs_], start=True, stop=True)
                        Hq = p2s.tile([128, 4 * C], bf16, tag="Hq",
                                      name=f"Hq{ci}_{lev + 1}")
                        nc.scalar.copy(Hq[:], h2q[:])

                ubs = []
                for h in range(4):
                    ub = p2s.tile([128, C], bf16, tag=f"ub{h}",
                                  name=f"ub{h}_{ci}")
                    nc.vector.tensor_scalar(ub[:], xaccs[h],
                                            beta2[:, h:h + 1], None,
                                            op0=Alu.mult)
                    ubs.append(ub)
                otq = q32(f"otq{ci}")
                ktq = q16(f"ktq{ci}")
                for h in range(4):
                    hs_ = slice(h * C, (h + 1) * C)
                    nc.tensor.matmul(otq[:, hs_], Sb[h][:], qtT[h][:],
                                     start=True, stop=False)
                    nc.tensor.matmul(otq[:, hs_], ubs[h][:],
                                     Gq[:, hs_], start=False, stop=True)
                    nc.tensor.transpose(ktq[:, hs_], kend[h][:], idb[:])
                ktsq = p2s.tile([128, 4 * C], bf16, tag="ktsq",
                                name=f"ktsq{ci}")
                nc.scalar.copy(ktsq[:], ktq[:])
                suq = q32(f"suq{ci}")
                for h in range(4):
                    hs_ = slice(h * C, (h + 1) * C)
                    nc.tensor.matmul(suq[:, hs_], ktsq[:, hs_], ubs[h][:],
                                     start=True, stop=True)
                    nc.vector.scalar_tensor_tensor(Sf[h][:], Sf[h][:],
                                                   bCq[:, h:h + 1],
                                                   suq[:, hs_],
                                                   op0=Alu.mult, op1=Alu.add)
                    nc.scalar.copy(Sb[h][:], Sf[h][:])
                sspq = None
                for h in range(4):
                    hs_ = slice(h * C, (h + 1) * C)
                    yf = yfall[h]
                    nc.vector.tensor_tensor(yf[:, ts], gateb[h][:, ts],
                                            otq[:, hs_], op=Alu.mult)
                    ysq = p2s.tile([128, C], bf16, tag=f"ysq{h}",
                                   name=f"ysq{h}_{ci}")
                    nc.gpsimd.tensor_tensor(ysq[:], yf[:, ts], yf[:, ts],
                                            op=Alu.mult)
                    if h == 0:
                        sspq = pr.tile([128, 4 * C], fp32, tag="xaq", bufs=1,
                                       name=f"ssp{ci}")
                    nc.tensor.matmul(sspq[0:4, 0:C], oh4t[:, 4 * h:4 * h + 4],
                                     ysq[:], start=(h == 0), stop=(h == 3))
                    if h == 3:
                        nc.scalar.copy(sqs[:, ts], sspq[0:4, 0:C])
                if sub_next:
                    sub_next.step(3)

            ax_prev = stage_a(0)
            for ci in range(NCH):
                sub_next = stage_a_subs(ci + 1) if ci + 1 < NCH else []
                stage_b(ax_prev, sub_next)
                ax_prev = sub_next.pop() if sub_next else None

        # =========== PHASE 3: gated RMS norm + output projection ===========
        with tc.tile_pool(name="po", bufs=1, space="PSUM") as po:
            nrm4 = chk.tile([4, T], fp32, tag="nrm4", name="nrm4")
            nc.scalar.activation(nrm4[:], sqs[:], Act.Ln, scale=1.0 / DV,
                                 bias=eps4[:, 0:1])
            rst4 = chk.tile([4, T], bf16, tag="rst4", name="rst4")
            nc.scalar.activation(rst4[:], nrm4[:], Act.Exp, scale=-0.5)
            for half in range(2):
                sl = slice(half * 512, (half + 1) * 512)
                for h in range(4):
                    rbc = po.tile([128, 512], fp32, tag="rbc", bufs=2,
                                  name=f"rbc{h}_{half}")
                    nc.tensor.matmul(rbc[:], oh4bt[:, h * 128:(h + 1) * 128],
                                     rst4[:, sl], start=True, stop=True)
                    nc.vector.scalar_tensor_tensor(yfall[h][:, sl],
                                                   yfall[h][:, sl],
                                                   nwt[:, 0:1], rbc[:],
                                                   op0=Alu.mult, op1=Alu.mult)
                for m in range(16):
                    ps = po.tile([128, 512], fp32, tag="ops", bufs=3,
                                 name="ops")
                    for k in range(4):
                        nc.tensor.matmul(ps[:], wot[k][:, m * 128:(m + 1) * 128],
                                         yfall[k][:, sl],
                                         start=(k == 0), stop=(k == 3))
                    osb = convp.tile([128, 512], fp32, tag="osb", name="osb",
                                     bufs=3)
                    if m % 2 == 0:
                        nc.vector.tensor_copy(osb[:], ps[:])
                    else:
                        nc.scalar.copy(osb[:], ps[:])
                    dma(outT[m * 128:(m + 1) * 128, sl], osb[:])

    nc.compile()
    return nc


def _prep_inputs(inputs):
    f32 = np.float32
    hs = np.asarray(inputs['hidden_states'], f32)
    maps = []
    tri = np.tril(np.ones((C, C), f32))
    nmaskM = (-(1.0 - tri)).astype(BF)                      # -1 strictly upper
    maskG = (1.0 - tri + np.eye(C, dtype=f32)).astype(BF)   # +1 upper incl diag
    repl = np.zeros((NG, DK), f32)
    for n in range(NG):
        repl[n, n * GG:(n + 1) * GG] = 1.0
    sel8 = np.zeros((NG, NG * 128), f32)
    for n in range(NG):
        sel8[n, n * 128:(n + 1) * 128] = 1.0
    oh8 = np.zeros((DK, 64), f32)
    for i in range(8):
        oh8[:, i * 8 + i] = 1.0
    oh4 = np.zeros((DK, 16), f32)
    for i in range(4):
        oh4[:, i * 4 + i] = 1.0
    oh4b = np.zeros((4, 4 * DK), f32)
    for i in range(4):
        oh4b[i, i * 128:(i + 1) * 128] = 1.0
    evod = np.zeros((DK, 4), f32)
    for cc in range(DK):
        evod[cc, (cc // GG) % 4] = 1.0
    ident = np.eye(128, dtype=f32)
    hTs = [np.ascontiguousarray(hs[b].T).astype(BF) for b in range(B)]
    for c in range(8):
        b, hg = c // 4, c % 4
        cols = slice(hg * NH * DK, (hg + 1) * NH * DK)
        gcols = slice(hg * NH * NG, (hg + 1) * NH * NG)
        hcols = slice(hg * NH, (hg + 1) * NH)
        nega = np.tile(np.exp(np.asarray(inputs['A_log'], f32)[hcols])[None, :],
                       (NG, 1))
        m = {
            'hT': hTs[b],
            'wqkvg': np.ascontiguousarray(np.concatenate(
                [np.asarray(inputs['Wq'], f32)[:, cols],
                 np.asarray(inputs['Wk'], f32)[:, cols],
                 np.asarray(inputs['Wv'], f32)[:, cols],
                 np.asarray(inputs['Wg'], f32)[:, cols]], 1)).astype(BF),
            'wo': np.asarray(inputs['Wo'], f32)[cols, :].astype(BF),
            'wf1': np.asarray(inputs['Wf1'], f32).astype(BF),
            'wf2': np.asarray(inputs['Wf2'], f32)[:, gcols].astype(BF),
            'wb': np.asarray(inputs['Wb'], f32)[:, hcols].astype(BF),
            'cw': np.ascontiguousarray(np.concatenate(
                [np.asarray(inputs['conv_q'], f32)[cols],
                 np.asarray(inputs['conv_k'], f32)[cols],
                 np.asarray(inputs['conv_v'], f32)[cols]], 1)),
            'nega': np.ascontiguousarray(nega).astype(f32),
            'dtb': np.ascontiguousarray(
                np.asarray(inputs['dt_bias'], f32)[gcols].reshape(NH, NG).T),
            'bgc': np.ascontiguousarray(
                np.asarray(inputs['bg'], f32)[cols].reshape(NH, DV).T),
            'normw': np.ascontiguousarray(
                np.asarray(inputs['norm_w'], f32)[:, None]),
            'repl': repl.astype(BF),
            'self8f': sel8.astype(BF),
            'sel8b': sel8.astype(BF),
            'onescol': np.ones((DK, 1), f32).astype(BF),
            'oh4': oh4.astype(BF),
            'oh4b': oh4b.astype(BF),
            'evod': evod,
            'oh8': oh8.astype(BF),
            'sc8': np.array([[1.0 / SCALE ** 2]] * 4 + [[1.0]] * 4, f32),
            'eps8': np.array([[1e-6 / SCALE ** 2]] * 4 + [[1e-6]] * 4, f32),
            'nmaskM': nmaskM,
            'maskG': maskG,
            'idbf': ident.astype(BF),
            'id4f': np.tile(ident, (1, 4)).astype(BF),
            'idf32': ident,
        }
        maps.append(m)
    return maps


def kernel(**inputs):
    from concourse.bass_utils import run_bass_kernel_spmd
    if 'nc' not in _CACHE:
        _CACHE['nc'] = _build()
    nc = _CACHE['nc']
    maps = _prep_inputs(inputs)
    res = run_bass_kernel_spmd(nc, maps, list(range(8))).results
    out = np.zeros((B, T, D), np.float32)
    for c in range(8):
        out[c // 4] += res[c]['outT'].T.astype(np.float32)
    return out
